# revision 33
# baseline (speedup 1.0000x reference)
"""Trainium2 Bass kernel for nn_MemoryRetriever (cross-attention memory retriever).

Strategy (v2):
- Host-side mask compaction: masked-out keys (~50%) are dropped on the host;
  survivors are dealt evenly to the 8 cores (n_c ~ Nkeep/8 each, zero-padded
  to SKC keys/core, pads confined to each core's last 512-key chunk and
  neutralized by a -224 additive bias folded into the scores matmul).
- fp8e4 DoubleRow matmuls (0.5 cycles/row) for Q/K/V projections, scores,
  attention*V and the denominator reduction.  Weights are pre-scaled by 16 on
  the host (fp8e4 subnormal floor) and descaled on PSUM read-out.
- Scores carry the mask bias inside the same DoubleRow instruction: the
  stationary operand's second half holds the bias row (partition 0), the
  moving operand's second half is a one-hot row, so exp() needs no per-tile
  bias and fuses across tiles.
- Engine balance: PE does all matmuls; Act does exp + rms (ln/exp pair);
  DVE does rope muls; GPSIMD (Pool) does PSUM->SBUF descale copies and
  broadcasts; numerator/denominator PSUM banks are DMA'd straight to DRAM.
- One AllReduce of [1032, 512] fp32 partial numerators/denominators, then
  each core output-projects its own 64-query slice.
"""

import os
import sys
import numpy as np

sys.path.insert(0, "/opt/trn_rl_repo")

DIM = 1024
HEADS = 8
HD = 128
SQ = 512
N_CORES = 8
QS = SQ // N_CORES
EPS = 1e-6
SCALE = 1.0 / np.sqrt(128.0)
WS = 16.0            # host-side weight scale (fp8 subnormal avoidance)
SHIFT = -3.0         # exp(score + SHIFT): keeps fp8e4 pt in range
MBPAD = -224.0       # additive raw-score bias for padded keys (pre-SCALE)
CHT = 4              # key tiles (128) per chunk

_cache = {}


def _build(skc=2048):
    key = ("nc", skc)
    if key in _cache:
        return _cache[key]

    import concourse.bass as bass
    import concourse.tile as tile
    from concourse import mybir, bacc

    f32 = mybir.dt.float32
    bf16 = mybir.dt.bfloat16
    fp8 = mybir.dt.float8e4
    AF = mybir.ActivationFunctionType
    DR = mybir.MatmulPerfMode.DoubleRow

    TT = skc // 128          # key tiles per core
    NCH = TT // CHT          # chunks per core
    assert skc % (CHT * 128) == 0

    _sim = os.environ.get("KSIM", "0") == "1"

    nc = bacc.Bacc("TRN2", target_bir_lowering=False, debug=False,
                   num_devices=N_CORES)

    def din(name, shape, dt=f32):
        return nc.dram_tensor(name, list(shape), dt, kind="ExternalInput").ap()

    # per-core sharded inputs
    memT = din("memT", [DIM, skc], fp8)       # compacted mem shard, feature-major
    cstk = din("cstk", [HD, 2, skc], bf16)    # K rope cos/sin (pair-dup rows)
    mbr = din("mbr", [1, HEADS, skc], fp8)    # 0 real / MBPAD pad rows
    # shared inputs
    xt = din("xt", [128, 8, SQ], fp8)         # x.T tiled [p,i,q]
    wq = din("wq", [128, 8, 8, 128], fp8)     # [p,i,o,m] = WS*Wq.T[i*128+p, o*128+m]
    wk = din("wk", [128, 8, 8, 128], fp8)
    wo = din("wo", [128, 8, 8, 128], fp8)
    wv = din("wv", [128, 8, DIM], fp8)        # [p,i,d] = WS*Wv.T[i*128+p, d]
    ctq = din("ctq", [128, 8, SQ], bf16)      # q rope cos (gq*gk folded)
    stq = din("stq", [128, 8, SQ], bf16)
    bq_t = din("bq_t", [128, 8])
    bk_t = din("bk_t", [128, 8])
    bo_t = din("bo_t", [128, 8])              # bo + Wo@bv folded
    pmat = din("pmat", [128, 128], bf16)      # P.T for rope pair swap (+-1)
    ones_c = din("ones_c", [128, 1], bf16)
    sel = din("sel", [128, 2, 64], fp8)       # den head selector
    qones = din("qones", [1, 8, SQ], fp8)     # one-hot rows for qT_dr
    eps_c = din("eps_c", [1, 1])
    shf_c = din("shf_c", [128, 1])

    outT = nc.dram_tensor("outT", [DIM, SQ], f32, kind="ExternalOutput").ap()
    cat = nc.dram_tensor("cat", [DIM + HEADS, SQ], bf16)
    cat_sh = nc.dram_tensor("cat_sh", [DIM + HEADS, SQ], bf16,
                            addr_space="Shared")

    MUL = mybir.AluOpType.mult
    ADD = mybir.AluOpType.add

    with tile.TileContext(nc) as tc:
        ctx_pools = []   # list of (cm, entered)

        def pool(name, bufs, space=None):
            kw = dict(name=name, bufs=bufs)
            if space:
                kw["space"] = space
            cm = tc.tile_pool(**kw)
            entered = cm.__enter__()
            ctx_pools.append((cm, entered))
            return entered

        def close_pool(entered):
            for i, (cm, e) in enumerate(ctx_pools):
                if e is entered:
                    cm.__exit__(None, None, None)
                    ctx_pools.pop(i)
                    return
            raise KeyError("pool not found")

        consts = pool("consts", 1)
        resid = pool("resid", 1)
        pp_den = pool("pp_den", 1, space="PSUM")  # den [8,512]
        pp_s = pool("pp_s", 1, space="PSUM")      # swap + scores [128,2,512]

        # ---- constants / resident tensors ----
        _cnt = [0]

        def cload(shape, dt, src, via=nc.sync):
            _cnt[0] += 1
            t = consts.tile(shape, dt, tag=f"c{_cnt[0]}")
            via.dma_start(t[:], src)
            return t

        pt_s = cload([128, 128], bf16, pmat, via=nc.gpsimd)
        ones_s = cload([128, 1], bf16, ones_c, via=nc.gpsimd)
        sel_s = cload([128, 2, 64], fp8, sel, via=nc.gpsimd)
        bq_s = cload([128, 8], f32, bq_t, via=nc.gpsimd)
        bk_s = cload([128, 8], f32, bk_t, via=nc.gpsimd)
        bo_s = cload([128, 8], f32, bo_t, via=nc.gpsimd)
        wq_s = cload([128, 8, 8, 128], fp8, wq)
        eps_s = cload([1, 1], f32, eps_c, via=nc.gpsimd)
        shf_s = cload([128, 1], f32, shf_c, via=nc.gpsimd)

        qT = resid.tile([128, 8, 2, SQ], fp8)       # [d, h, dr-half, q]
        kra = resid.tile([128, 8, CHT, 2, 128], fp8)
        krb = resid.tile([128, 8, CHT, 2, 128], fp8)
        pt_all = resid.tile([128, 8, TT, SQ], fp8)  # exp(scores+shift)
        v_sb = resid.tile([128, TT, DIM], fp8)


        # zero the DoubleRow second halves (Act), then row DMAs fill biases
        nc.scalar.memzero(qT[:, :, 1, :])
        nc.gpsimd.dma_start(qT[0:1, :, 1, :], qones)
        nc.scalar.memzero(kra[:, :, :, 1, :])
        nc.vector.memset(krb[:, :, :, 1, :], 0.0)

        den_ps = pp_den.tile([8, SQ], f32)
        nacc = resid.tile([128, 8, SQ], bf16)
        dacc = resid.tile([8, SQ], bf16)

        wpool = pool("wpool", 2)    # small working tiles (ysq/ykn/yc/ys)
        spool = pool("spool", 1)    # [1,n] rs scalars
        pp_all = pool("pp_all", 2, space="PSUM")   # [128,512] proj/V/swap psum
        pp_sq2 = pool("pp_sq2", 1, space="PSUM")   # sumsq [1,512]
        kpool = pool("kpool", 2)

        POW = mybir.AluOpType.pow

        def rs_broadcast(ps_sq, n):
            """rs = (sumsq_raw/(DIM*WS^2) + eps)^-0.5 via GPSIMD pow + bcast.
            yk is kept raw (x WS); the WS fold here normalizes it away."""
            t = spool.tile([1, n], f32, tag="lnm")
            nc.vector.tensor_scalar(t[:], ps_sq[:], 1.0 / DIM,
                                    EPS * WS * WS, MUL, ADD)
            rs = spool.tile([1, n], bf16, tag="rs")
            nc.vector.tensor_scalar(rs[:], t[:], -0.5, 1.0, POW, MUL)
            rsb = wpool.tile([128, n], bf16, tag="rsb")
            nc.gpsimd.partition_broadcast(rsb[:], rs[:])
            return rsb

        def rope_thunks(ysrc, n, rsb_of, ct_of, st_of, out_half,
                        fold_tables=False):
            """per-o rope emission thunks (thunk 0 computes folded tables)."""
            box = {}

            def ro(o):
                if fold_tables:
                    if o == 0:
                        ct_r = wpool.tile([128, n], bf16, tag="ctr")
                        nc.vector.tensor_mul(ct_r[:], ct_of(0), rsb_of())
                        st_r = wpool.tile([128, n], bf16, tag="str")
                        nc.vector.tensor_mul(st_r[:], st_of(0), rsb_of())
                        box["ct"], box["st"] = ct_r, st_r
                    ykn, ct_o, st_o = ysrc[:, o, :], box["ct"][:], box["st"][:]
                else:
                    ykn = wpool.tile([128, n], bf16, tag="ykn")
                    nc.vector.tensor_mul(ykn[:], ysrc[:, o, :], rsb_of())
                    ct_o, st_o = ct_of(o), st_of(o)
                ys = wpool.tile([128, n], bf16, tag="ys")
                nc.vector.tensor_mul(ys[:], ykn, st_o)
                swp = pp_all.tile([128, n], f32, tag="pp")
                nc.tensor.matmul(swp[:], pt_s[:], ys[:])
                yc = wpool.tile([128, n], bf16, tag="yc")
                nc.vector.tensor_mul(yc[:], ykn, ct_o)
                nc.vector.tensor_add(out_half(o), yc[:], swp[:])

            return [lambda o=o: ro(o) for o in range(8)]

        def weave(*lanes):
            """emit lanes with proportional progress (round-robin)."""
            lanes = [list(ln) for ln in lanes if ln]
            total = sum(len(ln) for ln in lanes)
            idx = [0] * len(lanes)
            for step in range(1, total + 1):
                for li, ln in enumerate(lanes):
                    want = (step * len(ln) + total - 1) // total
                    while idx[li] < min(want, len(ln)):
                        ln[idx[li]]()
                        idx[li] += 1

        def unit_K(w_s, b_s, src, ydst, o, ps_sq):
            """one output-block projection + descale + sumsq contribution."""
            ps = pp_all.tile([128, 512], f32, tag="pp")
            for pr in range(4):
                nc.tensor.matmul(ps[:], w_s[:, 2 * pr:2 * pr + 2, o, :],
                                 src[:, 2 * pr:2 * pr + 2, :],
                                 start=(pr == 0), stop=(pr == 3), perf_mode=DR)
            nc.gpsimd.tensor_copy(ydst[:, o, :], ps[:])
            ysq = wpool.tile([128, 512], bf16, tag="ysq")
            nc.vector.tensor_mul(ysq[:], ydst[:, o, :], ydst[:, o, :])
            nc.tensor.matmul(ps_sq[:], ones_s[:], ysq[:],
                             start=(o == 0), stop=(o == 7))

        def unit_V(memt, gt, t):
            """V projection for key tile t of the current chunk."""
            for oh in range(2):
                ps = pp_all.tile([128, 512], f32, tag="pp")
                for pr in range(4):
                    nc.tensor.matmul(
                        ps[:],
                        memt[:, 2 * pr:2 * pr + 2, t * 128:(t + 1) * 128],
                        wv_s[:, 2 * pr:2 * pr + 2, oh * 512:(oh + 1) * 512],
                        start=(pr == 0), stop=(pr == 3), perf_mode=DR)
                if oh == 0:
                    nc.scalar.activation(v_sb[:, gt, oh * 512:(oh + 1) * 512],
                                         ps[:], AF.Copy)
                else:
                    nc.gpsimd.tensor_copy(
                        v_sb[:, gt, oh * 512:(oh + 1) * 512], ps[:])

        def group_SE(c, kr, h, p2=None):
            """scores + fused exp for head h over all 4 tiles of chunk c."""
            ps_s = pp_s.tile([128, CHT, 512], f32, tag="ps_s")
            for tt in range(CHT):
                nc.tensor.matmul(ps_s[:, tt, :], kr[:, h, tt, :, :],
                                 qT[:, h, :, :], perf_mode=DR)
            ptt = pt_all[:, h, c * CHT:(c + 1) * CHT, :]
            nc.scalar.activation(ptt, ps_s[:], AF.Exp,
                                 scale=SCALE, bias=shf_s[:])

        def den_mm(c, h, p2):
            gp = c * 2 + p2
            nc.tensor.matmul(den_ps[:], sel_s[:, :, h * 8:h * 8 + 8],
                             pt_all[:, h, gp * 2:gp * 2 + 2, :], perf_mode=DR,
                             start=(c == 0 and p2 == 0 and h == 0),
                             stop=(c == NCH - 1 and p2 == 1 and h == 7))

        def numer(h, pp):
            ps_n = pp.tile([128, SQ], f32, tag="pp")
            for p in range(TT // 2):
                nc.tensor.matmul(ps_n[:],
                                 v_sb[:, 2 * p:2 * p + 2, h * 128:(h + 1) * 128],
                                 pt_all[:, h, 2 * p:2 * p + 2, :],
                                 start=(p == 0), stop=(p == TT // 2 - 1),
                                 perf_mode=DR)
            nc.vector.tensor_copy(nacc[:, h, :], ps_n[:])
            nc.gpsimd.dma_start(
                cat[h * 128:(h + 1) * 128, :].rearrange("(a p) q -> p a q", p=128),
                nacc[:, h:h + 1, :])

        # =========== Q phase ===========
        qpool = pool("qpool", 1)
        ctq_s = qpool.tile([128, 8, SQ], bf16, tag="ctq")
        nc.gpsimd.dma_start(ctq_s[:], ctq)
        stq_s = qpool.tile([128, 8, SQ], bf16, tag="stq")
        nc.gpsimd.dma_start(stq_s[:], stq)
        xt_s = kpool.tile([128, 8, SQ], fp8, tag="memt")
        nc.sync.dma_start(xt_s[:], xt)
        wk_s = cload([128, 8, 8, 128], fp8, wk)
        wv_s = cload([128, 8, DIM], fp8, wv, via=nc.gpsimd)
        wo_s = cload([128, 8, 8, 128], fp8, wo, via=nc.gpsimd)  # late-needed
        # =========== pipelined chunk loop (Q phase = prologue) ===========
        cw = CHT * 128
        st = {}

        def s1_load(c):
            c0 = c * cw
            memt = kpool.tile([128, 8, cw], fp8, tag="memt")
            nc.sync.dma_start(
                memt[:], memT[:, c0:c0 + cw].rearrange("(i p) t -> p i t", p=128))
            cs_t = kpool.tile([128, 2, cw], bf16, tag="cstk")
            nc.sync.dma_start(cs_t[:], cstk[:, :, c0:c0 + cw])
            kr = kra if c % 2 == 0 else krb
            nc.sync.dma_start(
                kr[0:1, :, :, 1, :],
                mbr[0:1, :, c0:c0 + cw].rearrange("a o (t m) -> a o t m", m=128))
            return dict(memt=memt, ctk=cs_t[:, 0, :], stk=cs_t[:, 1, :], kr=kr)

        def chunk_units(c):
            st[c] = s1_load(c)
            ps_sq = pp_sq2.tile([1, cw], f32, tag="pssq")
            st[c]["ps_sq"] = ps_sq
            ykt = kpool.tile([128, 8, 512], bf16, tag="yk")
            st[c]["yk"] = ykt
            units = []
            for o in range(8):
                units.append(lambda o=o, c=c: unit_K(
                    wk_s, bk_s, st[c]["memt"], st[c]["yk"], o, st[c]["ps_sq"]))
                if o % 2 == 1:
                    units.append(lambda o=o, c=c: unit_V(
                        st[c]["memt"], c * CHT + o // 2, o // 2))
            return units

        def chunk_rope_thunks(c):
            kr = st[c]["kr"]
            return rope_thunks(
                st[c]["yk"], cw, lambda c=c: st[c]["rsb"][:],
                lambda o, c=c: st[c]["ctk"][:],
                lambda o, c=c: st[c]["stk"][:],
                lambda o, kr=kr: kr[:, o, :, 0, :], fold_tables=True)

        # Q prologue: Q proj, then Q rope woven with chunk-0 proj
        ps_sqq = pp_sq2.tile([1, SQ], f32, tag="pssq")
        yq = kpool.tile([128, 8, SQ], bf16, tag="yk")
        units0 = chunk_units(0)
        for o in range(8):
            unit_K(wq_s, bq_s, xt_s, yq, o, ps_sqq)
        rsb_q = rs_broadcast(ps_sqq, SQ)
        qrope = rope_thunks(yq, SQ, lambda: rsb_q[:],
                            lambda o: ctq_s[:, o, :], lambda o: stq_s[:, o, :],
                            lambda o: qT[:, o, 0, :])
        weave(units0, qrope)
        close_pool(qpool)
        st[0]["rsb"] = rs_broadcast(st[0]["ps_sq"], cw)

        # pipelined chunk stream: rope(c,o) -> scores/exp(c,h=o-1) -> dens
        # (lagged) with next chunk's proj units spread throughout
        for c in range(NCH):
            last = c == NCH - 1
            ropes = chunk_rope_thunks(c)
            units = chunk_units(c + 1) if not last else []
            kr = kra if c % 2 == 0 else krb
            denq = []          # lagged den emission queue
            ui = 0

            def unit_step(frac, n_slots=12):
                nonlocal ui
                want = min(len(units), (frac * len(units)) // n_slots + 1)
                while ui < want:
                    units[ui]()
                    ui += 1

            slot = 0
            for o in range(8):
                unit_step(slot)
                ropes[o]()
                slot += 1
                for h in ([o - 1] if o >= 1 else []):
                    group_SE(c, kr, h)
                    for p2 in range(2):
                        denq.append((c, h, p2))
                    while len(denq) > 2:
                        den_mm(*denq.pop(0))
                    if last:
                        numer(h, pp_all)
                    unit_step(slot)
                    slot += 1
            for h in (7,):
                group_SE(c, kr, h)
                for p2 in range(2):
                    denq.append((c, h, p2))
                while len(denq) > 2:
                    den_mm(*denq.pop(0))
                if last:
                    numer(h, pp_all)
                unit_step(slot)
                slot += 1
            while ui < len(units):
                units[ui]()
                ui += 1
            while denq:
                den_mm(*denq.pop(0))
            if not last:
                st[c + 1]["rsb"] = rs_broadcast(st[c + 1]["ps_sq"], cw)

        for p in (kpool, pp_sq2, pp_all, spool, wpool, pp_s):
            close_pool(p)

        nc.scalar.activation(dacc[:], den_ps[:], AF.Copy)
        nc.gpsimd.dma_start(cat[DIM:DIM + HEADS, :], dacc[:])

        # =========== reduce across cores ===========
        if _sim:
            nc.gpsimd.dma_start(cat_sh[:], cat[:])
        else:
            nc.gpsimd.collective_compute(
                "AllReduce", mybir.AluOpType.add,
                replica_groups=[list(range(N_CORES))],
                ins=[cat[:]], outs=[cat_sh[:]])

        # =========== per-core output projection on its query slice ==========
        tail = pool("tail", 1)
        nred = tail.tile([128, 8, QS], bf16)
        dred = tail.tile([1, HEADS, QS], bf16)
        pid = nc.sync.partition_id()
        qoff = pid * QS
        nc.sync.dma_start(
            nred[:],
            cat_sh[0:DIM, bass.ds(qoff, QS)].rearrange("(h p) q -> p h q", p=128))
        nc.sync.dma_start(dred[:], cat_sh[DIM:DIM + HEADS, bass.ds(qoff, QS)])
        rd = tail.tile([1, HEADS, QS], f32)
        nc.vector.reciprocal(rd[:], dred[:])
        rdb = tail.tile([128, HEADS, QS], f32)
        nc.gpsimd.partition_broadcast(rdb[:], rd[:])
        nsc = tail.tile([128, 8, QS], fp8)
        nc.vector.tensor_mul(nsc[:], nred[:], rdb[:])
        out_sb = tail.tile([128, 8, QS], f32)
        pp_t = pool("pp_t", 2, space="PSUM")
        for e in range(8):
            ps_o = pp_t.tile([128, QS], f32, tag="ppo")
            for pr in range(4):
                nc.tensor.matmul(ps_o[:], wo_s[:, 2 * pr:2 * pr + 2, e, :],
                                 nsc[:, 2 * pr:2 * pr + 2, :],
                                 start=(pr == 0), stop=(pr == 3), perf_mode=DR)
            nc.scalar.activation(out_sb[:, e, :], ps_o[:], AF.Identity,
                                 scale=1.0 / (WS * WS), bias=bo_s[:, e:e + 1])
        nc.sync.dma_start(
            outT.rearrange("(e p) q -> p e q", p=128)[:, :, 0:QS], out_sb[:])

        for cm, _ in reversed(ctx_pools):
            cm.__exit__(None, None, None)

    nc.compile()
    _cache[key] = nc
    _cache["nc"] = nc
    return nc


def _skc_for(nkeep):
    return max(CHT * 128, int(np.ceil(nkeep / (N_CORES * 512))) * 512)


def _prep(x, mem, mask, cos_q, sin_q, cos_k, sin_k,
          Wq, bq, Wk, bk, Wv, bv, Wo, bo, gq, gk):
    import ml_dtypes
    f = np.float32
    bf = ml_dtypes.bfloat16
    f8 = ml_dtypes.float8_e4m3
    x = np.asarray(x, f).reshape(SQ, DIM)
    mem = np.asarray(mem, f).reshape(-1, DIM)
    mask = np.asarray(mask).reshape(-1)
    cos_q = np.asarray(cos_q, f)
    sin_q = np.asarray(sin_q, f)
    cos_k = np.asarray(cos_k, f)
    sin_k = np.asarray(sin_k, f)
    Wq, Wk, Wv, Wo = (np.asarray(w, f) for w in (Wq, Wk, Wv, Wo))
    bq, bk, bv, bo, gq, gk = (np.asarray(v, f) for v in (bq, bk, bv, bo, gq, gk))

    if not np.allclose(gk, 1.0):
        gkp = gk.reshape(-1, 2)
        assert np.allclose(gkp[:, 0], gkp[:, 1]), "unsupported non-pairwise gk"

    idx = np.flatnonzero(mask)
    nkeep = len(idx)
    skc = _skc_for(nkeep)
    base, rem = divmod(nkeep, N_CORES)
    counts = [base + (1 if c < rem else 0) for c in range(N_CORES)]
    offs = np.concatenate([[0], np.cumsum(counts)])

    def tile_w(WT):  # [1024,1024] (in,out of W.T) -> [p, i, o, m], scaled
        return np.ascontiguousarray(
            (WT * WS).reshape(8, 128, 8, 128).transpose(1, 0, 2, 3)).astype(f8)

    ii = np.arange(128)
    jj = ii // 2
    partner = ii ^ 1

    # fold gq (and pairwise gk) into the q rope tables; sin pairs with
    # partner's gq
    gq_t = (gq * gk).reshape(8, 128)
    gq_sin = (gq.reshape(8, 128)[:, partner] * gk.reshape(8, 128))
    cq = cos_q[:, jj].T                # [128, SQ]
    sq = sin_q[:, jj].T
    ctq = np.ascontiguousarray(
        (cq[None, :, :] * gq_t[:, :, None]).transpose(1, 0, 2)).astype(bf)
    stq = np.ascontiguousarray(
        (sq[None, :, :] * gq_sin[:, :, None]).transpose(1, 0, 2)).astype(bf)

    PT = np.zeros((128, 128), f)
    even = ii[ii % 2 == 0]
    PT[even + 1, even] = -1.0
    PT[even, even + 1] = 1.0

    selm = np.zeros((128, 2, 64), f)
    for h in range(8):
        selm[:, :, h * 8 + h] = 1.0

    qones = np.ones((1, 8, SQ), f).astype(f8)

    bo_f = bo + Wo @ bv

    shared = {
        "xt": np.ascontiguousarray(
            x.T.reshape(8, 128, SQ).transpose(1, 0, 2)).astype(f8),
        "wq": tile_w(Wq.T), "wk": tile_w(Wk.T), "wo": tile_w(Wo.T),
        "wv": np.ascontiguousarray(
            (Wv.T * WS).reshape(8, 128, DIM).transpose(1, 0, 2)).astype(f8),
        "ctq": ctq, "stq": stq,
        "bq_t": np.ascontiguousarray(bq.reshape(8, 128).T),
        "bk_t": np.ascontiguousarray(bk.reshape(8, 128).T),
        "bo_t": np.ascontiguousarray(bo_f.reshape(8, 128).T),
        "pmat": PT.astype(bf),
        "ones_c": np.ones((128, 1), bf),
        "sel": selm.astype(f8),
        "qones": qones,
        "eps_c": np.full((1, 1), EPS, f),
        "shf_c": np.full((128, 1), SHIFT, f),
    }

    ckT = cos_k[:, jj].T.astype(f)     # [128, SK]
    skT = sin_k[:, jj].T.astype(f)

    in_maps = []
    for c in range(N_CORES):
        keys = idx[offs[c]:offs[c + 1]]
        n = len(keys)
        m = dict(shared)
        memc = np.zeros((DIM, skc), f8)
        memc[:, :n] = mem[keys].T.astype(f8)
        cstk = np.zeros((HD, 2, skc), bf)
        cstk[:, 0, :n] = ckT[:, keys].astype(bf)
        cstk[:, 1, :n] = skT[:, keys].astype(bf)
        mb = np.full((skc,), MBPAD, f)
        mb[:n] = 0.0
        m["memT"] = memc
        m["cstk"] = cstk
        m["mbr"] = np.ascontiguousarray(
            np.broadcast_to(mb[None, None, :], (1, HEADS, skc))).astype(f8)
        in_maps.append(m)
    return in_maps


def _assemble(outTs):
    parts = [np.asarray(outTs[c])[:, 0:QS].T for c in range(N_CORES)]
    out = np.concatenate(parts, axis=0)
    return out[None].astype(np.float32)


def kernel(**inputs):
    from concourse.bass_utils import run_bass_kernel_spmd
    in_maps = _prep(**inputs)
    skc = in_maps[0]["memT"].shape[1]
    nc = _build(skc)
    res = run_bass_kernel_spmd(nc, in_maps, list(range(N_CORES)))
    return _assemble([res.results[c]["outT"] for c in range(N_CORES)])


# revision 34
# speedup vs baseline: 1.0777x; 1.0777x over previous
"""Trainium2 Bass kernel for nn_MemoryRetriever (cross-attention memory retriever).

Strategy (v2):
- Host-side mask compaction: masked-out keys (~50%) are dropped on the host;
  survivors are dealt evenly to the 8 cores (n_c ~ Nkeep/8 each, zero-padded
  to SKC keys/core, pads confined to each core's last 512-key chunk and
  neutralized by a -224 additive bias folded into the scores matmul).
- fp8e4 DoubleRow matmuls (0.5 cycles/row) for Q/K/V projections, scores,
  attention*V and the denominator reduction.  Weights are pre-scaled by 16 on
  the host (fp8e4 subnormal floor) and descaled on PSUM read-out.
- Scores carry the mask bias inside the same DoubleRow instruction: the
  stationary operand's second half holds the bias row (partition 0), the
  moving operand's second half is a one-hot row, so exp() needs no per-tile
  bias and fuses across tiles.
- Engine balance: PE does all matmuls; Act does exp + rms (ln/exp pair);
  DVE does rope muls; GPSIMD (Pool) does PSUM->SBUF descale copies and
  broadcasts; numerator/denominator PSUM banks are DMA'd straight to DRAM.
- One AllReduce of [1032, 512] fp32 partial numerators/denominators, then
  each core output-projects its own 64-query slice.
"""

import os
import sys
import numpy as np

sys.path.insert(0, "/opt/trn_rl_repo")

DIM = 1024
HEADS = 8
HD = 128
SQ = 512
N_CORES = 8
QS = SQ // N_CORES
EPS = 1e-6
SCALE = 1.0 / np.sqrt(128.0)
WS = 16.0            # host-side weight scale (fp8 subnormal avoidance)
SHIFT = -3.0         # exp(score + SHIFT): keeps fp8e4 pt in range
MBPAD = -224.0       # additive raw-score bias for padded keys (pre-SCALE)
CHT = 4              # key tiles (128) per chunk

_cache = {}


def _build(skc=2048):
    key = ("nc", skc)
    if key in _cache:
        return _cache[key]

    import concourse.bass as bass
    import concourse.tile as tile
    from concourse import mybir, bacc

    f32 = mybir.dt.float32
    bf16 = mybir.dt.bfloat16
    fp8 = mybir.dt.float8e4
    AF = mybir.ActivationFunctionType
    DR = mybir.MatmulPerfMode.DoubleRow

    TT = skc // 128          # key tiles per core
    NCH = TT // CHT          # chunks per core
    assert skc % (CHT * 128) == 0

    _sim = os.environ.get("KSIM", "0") == "1"

    nc = bacc.Bacc("TRN2", target_bir_lowering=False, debug=False,
                   num_devices=N_CORES)

    def din(name, shape, dt=f32):
        return nc.dram_tensor(name, list(shape), dt, kind="ExternalInput").ap()

    # per-core sharded inputs
    memT = din("memT", [DIM, skc], fp8)       # compacted mem shard, feature-major
    cstk = din("cstk", [HD, 2, skc], bf16)    # K rope cos/sin (pair-dup rows)
    mbr = din("mbr", [1, HEADS, skc], fp8)    # 0 real / MBPAD pad rows
    # shared inputs
    xt = din("xt", [128, 8, SQ], fp8)         # x.T tiled [p,i,q]
    wq = din("wq", [128, 8, 8, 128], fp8)     # [p,i,o,m] = WS*Wq.T[i*128+p, o*128+m]
    wk = din("wk", [128, 8, 8, 128], fp8)
    wo = din("wo", [128, 8, 8, 128], fp8)
    wv = din("wv", [128, 8, DIM], fp8)        # [p,i,d] = WS*Wv.T[i*128+p, d]
    ctq = din("ctq", [128, 8, SQ], bf16)      # q rope cos (gq*gk folded)
    stq = din("stq", [128, 8, SQ], bf16)
    bq_t = din("bq_t", [128, 8])
    bk_t = din("bk_t", [128, 8])
    bo_t = din("bo_t", [128, 8])              # bo + Wo@bv folded
    pmat = din("pmat", [128, 128], bf16)      # P.T for rope pair swap (+-1)
    ones_c = din("ones_c", [128, 1], bf16)
    sel = din("sel", [128, 2, 64], fp8)       # den head selector
    qones = din("qones", [1, 8, SQ], fp8)     # one-hot rows for qT_dr
    eps_c = din("eps_c", [1, 1])
    shf_c = din("shf_c", [128, 1])

    outT = nc.dram_tensor("outT", [DIM, SQ], f32, kind="ExternalOutput").ap()
    cat = nc.dram_tensor("cat", [DIM + HEADS, SQ], bf16)
    cat_sh = nc.dram_tensor("cat_sh", [DIM + HEADS, SQ], bf16,
                            addr_space="Shared")

    MUL = mybir.AluOpType.mult
    ADD = mybir.AluOpType.add

    with tile.TileContext(nc) as tc:
        ctx_pools = []   # list of (cm, entered)

        def pool(name, bufs, space=None):
            kw = dict(name=name, bufs=bufs)
            if space:
                kw["space"] = space
            cm = tc.tile_pool(**kw)
            entered = cm.__enter__()
            ctx_pools.append((cm, entered))
            return entered

        def close_pool(entered):
            for i, (cm, e) in enumerate(ctx_pools):
                if e is entered:
                    cm.__exit__(None, None, None)
                    ctx_pools.pop(i)
                    return
            raise KeyError("pool not found")

        consts = pool("consts", 1)
        resid = pool("resid", 1)
        pp_den = pool("pp_den", 1, space="PSUM")  # den [8,512]
        pp_s = pool("pp_s", 1, space="PSUM")      # swap + scores [128,2,512]

        # ---- constants / resident tensors ----
        _cnt = [0]

        def cload(shape, dt, src, via=nc.sync):
            _cnt[0] += 1
            t = consts.tile(shape, dt, tag=f"c{_cnt[0]}")
            via.dma_start(t[:], src)
            return t

        pt_s = cload([128, 128], bf16, pmat, via=nc.gpsimd)
        ones_s = cload([128, 1], bf16, ones_c, via=nc.gpsimd)
        sel_s = cload([128, 2, 64], fp8, sel, via=nc.gpsimd)
        bq_s = cload([128, 8], f32, bq_t, via=nc.gpsimd)
        bk_s = cload([128, 8], f32, bk_t, via=nc.gpsimd)
        bo_s = cload([128, 8], f32, bo_t, via=nc.gpsimd)
        wq_s = cload([128, 8, 8, 128], fp8, wq)
        eps_s = cload([1, 1], f32, eps_c, via=nc.gpsimd)
        shf_s = cload([128, 1], f32, shf_c, via=nc.gpsimd)

        qT = resid.tile([128, 8, 2, SQ], fp8)       # [d, h, dr-half, q]
        kra = resid.tile([128, 8, CHT, 2, 128], fp8)
        krb = resid.tile([128, 8, CHT, 2, 128], fp8)
        pt_all = resid.tile([128, 8, TT, SQ], fp8)  # exp(scores+shift)
        v_sb = resid.tile([128, TT, DIM], fp8)


        # zero the DoubleRow second halves (Act), then row DMAs fill biases
        nc.scalar.memzero(qT[:, :, 1, :])
        nc.gpsimd.dma_start(qT[0:1, :, 1, :], qones)
        nc.scalar.memzero(kra[:, :, :, 1, :])
        nc.vector.memset(krb[:, :, :, 1, :], 0.0)

        den_ps = pp_den.tile([8, SQ], f32)
        nacc = resid.tile([128, 8, SQ], bf16)
        dacc = resid.tile([8, SQ], bf16)

        wpool = pool("wpool", 2)    # small working tiles (ysq/ykn/yc/ys)
        spool = pool("spool", 1)    # [1,n] rs scalars
        pp_all = pool("pp_all", 2, space="PSUM")   # [128,512] proj/V/swap psum
        pp_sq2 = pool("pp_sq2", 1, space="PSUM")   # sumsq [1,512]
        kpool = pool("kpool", 2)

        POW = mybir.AluOpType.pow

        def rs_broadcast(ps_sq, n):
            """rs = (sumsq_raw/(DIM*WS^2) + eps)^-0.5 via GPSIMD pow + bcast.
            yk is kept raw (x WS); the WS fold here normalizes it away."""
            t = spool.tile([1, n], f32, tag="lnm")
            nc.vector.tensor_scalar(t[:], ps_sq[:], 1.0 / DIM,
                                    EPS * WS * WS, MUL, ADD)
            rs = spool.tile([1, n], bf16, tag="rs")
            nc.vector.tensor_scalar(rs[:], t[:], -0.5, 1.0, POW, MUL)
            rsb = wpool.tile([128, n], bf16, tag="rsb")
            nc.gpsimd.partition_broadcast(rsb[:], rs[:])
            return rsb

        def rope_thunks(ysrc, n, rsb_of, ct_of, st_of, out_half,
                        fold_tables=False):
            """per-o rope emission thunks (thunk 0 computes folded tables)."""
            box = {}

            def ro(o):
                if fold_tables:
                    if o == 0:
                        ct_r = wpool.tile([128, n], bf16, tag="ctr")
                        nc.vector.tensor_mul(ct_r[:], ct_of(0), rsb_of())
                        st_r = wpool.tile([128, n], bf16, tag="str")
                        nc.vector.tensor_mul(st_r[:], st_of(0), rsb_of())
                        box["ct"], box["st"] = ct_r, st_r
                    ykn, ct_o, st_o = ysrc[:, o, :], box["ct"][:], box["st"][:]
                else:
                    ykn = wpool.tile([128, n], bf16, tag="ykn")
                    nc.vector.tensor_mul(ykn[:], ysrc[:, o, :], rsb_of())
                    ct_o, st_o = ct_of(o), st_of(o)
                ys = wpool.tile([128, n], bf16, tag="ys")
                nc.vector.tensor_mul(ys[:], ykn, st_o)
                swp = pp_all.tile([128, n], f32, tag="pp")
                nc.tensor.matmul(swp[:], pt_s[:], ys[:])
                yc = wpool.tile([128, n], bf16, tag="yc")
                nc.vector.tensor_mul(yc[:], ykn, ct_o)
                nc.vector.tensor_add(out_half(o), yc[:], swp[:])

            return [lambda o=o: ro(o) for o in range(8)]

        def weave(*lanes):
            """emit lanes with proportional progress (round-robin)."""
            lanes = [list(ln) for ln in lanes if ln]
            total = sum(len(ln) for ln in lanes)
            idx = [0] * len(lanes)
            for step in range(1, total + 1):
                for li, ln in enumerate(lanes):
                    want = (step * len(ln) + total - 1) // total
                    while idx[li] < min(want, len(ln)):
                        ln[idx[li]]()
                        idx[li] += 1

        def unit_K(w_s, b_s, src, ydst, o, ps_sq):
            """one output-block projection + descale + sumsq contribution."""
            ps = pp_all.tile([128, 512], f32, tag="pp")
            for pr in range(4):
                nc.tensor.matmul(ps[:], w_s[:, 2 * pr:2 * pr + 2, o, :],
                                 src[:, 2 * pr:2 * pr + 2, :],
                                 start=(pr == 0), stop=(pr == 3), perf_mode=DR)
            nc.gpsimd.tensor_copy(ydst[:, o, :], ps[:])
            ysq = wpool.tile([128, 512], bf16, tag="ysq")
            nc.vector.tensor_mul(ysq[:], ydst[:, o, :], ydst[:, o, :])
            nc.tensor.matmul(ps_sq[:], ones_s[:], ysq[:],
                             start=(o == 0), stop=(o == 7))

        def unit_V(memt, gt, t):
            """V projection for key tile t of the current chunk."""
            for oh in range(2):
                ps = pp_all.tile([128, 512], f32, tag="pp")
                for pr in range(4):
                    nc.tensor.matmul(
                        ps[:],
                        memt[:, 2 * pr:2 * pr + 2, t * 128:(t + 1) * 128],
                        wv_s[:, 2 * pr:2 * pr + 2, oh * 512:(oh + 1) * 512],
                        start=(pr == 0), stop=(pr == 3), perf_mode=DR)
                if oh == 0:
                    nc.scalar.activation(v_sb[:, gt, oh * 512:(oh + 1) * 512],
                                         ps[:], AF.Copy)
                else:
                    nc.gpsimd.tensor_copy(
                        v_sb[:, gt, oh * 512:(oh + 1) * 512], ps[:])

        def group_SE(c, kr, h, p2):
            """scores + fused exp for (head h, tile pair p2) of chunk c."""
            ps_s = pp_s.tile([128, 2, 512], f32, tag="ps_s")
            for tt in range(2):
                nc.tensor.matmul(ps_s[:, tt, :], kr[:, h, p2 * 2 + tt, :, :],
                                 qT[:, h, :, :], perf_mode=DR)
            gp = c * 2 + p2
            ptt = pt_all[:, h, gp * 2:gp * 2 + 2, :]
            nc.scalar.activation(ptt, ps_s[:], AF.Exp,
                                 scale=SCALE, bias=shf_s[:])

        def den_mm(c, h, p2):
            gp = c * 2 + p2
            nc.tensor.matmul(den_ps[:], sel_s[:, :, h * 8:h * 8 + 8],
                             pt_all[:, h, gp * 2:gp * 2 + 2, :], perf_mode=DR,
                             start=(c == 0 and p2 == 0 and h == 0),
                             stop=(c == NCH - 1 and p2 == 1 and h == 7))

        def numer(h, pp):
            ps_n = pp.tile([128, SQ], f32, tag="pp")
            for p in range(TT // 2):
                nc.tensor.matmul(ps_n[:],
                                 v_sb[:, 2 * p:2 * p + 2, h * 128:(h + 1) * 128],
                                 pt_all[:, h, 2 * p:2 * p + 2, :],
                                 start=(p == 0), stop=(p == TT // 2 - 1),
                                 perf_mode=DR)
            nc.vector.tensor_copy(nacc[:, h, :], ps_n[:])
            nc.gpsimd.dma_start(
                cat[h * 128:(h + 1) * 128, :].rearrange("(a p) q -> p a q", p=128),
                nacc[:, h:h + 1, :])

        # =========== Q phase ===========
        qpool = pool("qpool", 1)
        ctq_s = qpool.tile([128, 8, SQ], bf16, tag="ctq")
        nc.gpsimd.dma_start(ctq_s[:], ctq)
        stq_s = qpool.tile([128, 8, SQ], bf16, tag="stq")
        nc.gpsimd.dma_start(stq_s[:], stq)
        xt_s = kpool.tile([128, 8, SQ], fp8, tag="memt")
        nc.sync.dma_start(xt_s[:], xt)
        wk_s = cload([128, 8, 8, 128], fp8, wk)
        wv_s = cload([128, 8, DIM], fp8, wv, via=nc.gpsimd)
        wo_s = cload([128, 8, 8, 128], fp8, wo, via=nc.gpsimd)  # late-needed
        # =========== pipelined chunk loop (Q phase = prologue) ===========
        cw = CHT * 128
        st = {}

        def s1_load(c):
            c0 = c * cw
            memt = kpool.tile([128, 8, cw], fp8, tag="memt")
            nc.sync.dma_start(
                memt[:], memT[:, c0:c0 + cw].rearrange("(i p) t -> p i t", p=128))
            cs_t = kpool.tile([128, 2, cw], bf16, tag="cstk")
            nc.sync.dma_start(cs_t[:], cstk[:, :, c0:c0 + cw])
            kr = kra if c % 2 == 0 else krb
            nc.sync.dma_start(
                kr[0:1, :, :, 1, :],
                mbr[0:1, :, c0:c0 + cw].rearrange("a o (t m) -> a o t m", m=128))
            return dict(memt=memt, ctk=cs_t[:, 0, :], stk=cs_t[:, 1, :], kr=kr)

        def chunk_units(c):
            st[c] = s1_load(c)
            ps_sq = pp_sq2.tile([1, cw], f32, tag="pssq")
            st[c]["ps_sq"] = ps_sq
            ykt = kpool.tile([128, 8, 512], bf16, tag="yk")
            st[c]["yk"] = ykt
            units = []
            for o in range(8):
                units.append(lambda o=o, c=c: unit_K(
                    wk_s, bk_s, st[c]["memt"], st[c]["yk"], o, st[c]["ps_sq"]))
                if o % 2 == 1:
                    units.append(lambda o=o, c=c: unit_V(
                        st[c]["memt"], c * CHT + o // 2, o // 2))
            return units

        def chunk_rope_thunks(c):
            kr = st[c]["kr"]
            return rope_thunks(
                st[c]["yk"], cw, lambda c=c: st[c]["rsb"][:],
                lambda o, c=c: st[c]["ctk"][:],
                lambda o, c=c: st[c]["stk"][:],
                lambda o, kr=kr: kr[:, o, :, 0, :], fold_tables=True)

        # Q prologue: Q proj, then Q rope woven with chunk-0 proj
        ps_sqq = pp_sq2.tile([1, SQ], f32, tag="pssq")
        yq = kpool.tile([128, 8, SQ], bf16, tag="yk")
        units0 = chunk_units(0)
        for o in range(8):
            unit_K(wq_s, bq_s, xt_s, yq, o, ps_sqq)
        rsb_q = rs_broadcast(ps_sqq, SQ)
        qrope = rope_thunks(yq, SQ, lambda: rsb_q[:],
                            lambda o: ctq_s[:, o, :], lambda o: stq_s[:, o, :],
                            lambda o: qT[:, o, 0, :])
        weave(units0, qrope)
        close_pool(qpool)
        st[0]["rsb"] = rs_broadcast(st[0]["ps_sq"], cw)

        # pipelined chunk stream: rope(c,o) -> scores/exp(c,h=o-1) -> dens
        # (lagged) with next chunk's proj units spread throughout
        for c in range(NCH):
            last = c == NCH - 1
            ropes = chunk_rope_thunks(c)
            units = chunk_units(c + 1) if not last else []
            kr = kra if c % 2 == 0 else krb
            denq = []          # lagged den emission queue
            ui = 0

            def unit_step(frac, n_slots=12):
                nonlocal ui
                want = min(len(units), (frac * len(units)) // n_slots + 1)
                while ui < want:
                    units[ui]()
                    ui += 1

            slot = 0
            for o in range(8):
                unit_step(slot)
                ropes[o]()
                slot += 1
                for h in ([o - 1] if o >= 1 else []):
                    for p2 in range(2):
                        group_SE(c, kr, h, p2)
                        denq.append((c, h, p2))
                        while len(denq) > 2:
                            den_mm(*denq.pop(0))
                    if last:
                        numer(h, pp_all)
                    unit_step(slot)
                    slot += 1
            for h in (7,):
                for p2 in range(2):
                    group_SE(c, kr, h, p2)
                    denq.append((c, h, p2))
                    while len(denq) > 2:
                        den_mm(*denq.pop(0))
                if last:
                    numer(h, pp_all)
                unit_step(slot)
                slot += 1
            while ui < len(units):
                units[ui]()
                ui += 1
            while denq:
                den_mm(*denq.pop(0))
            if not last:
                st[c + 1]["rsb"] = rs_broadcast(st[c + 1]["ps_sq"], cw)

        for p in (kpool, pp_sq2, pp_all, spool, wpool, pp_s):
            close_pool(p)

        nc.scalar.activation(dacc[:], den_ps[:], AF.Copy)
        nc.gpsimd.dma_start(cat[DIM:DIM + HEADS, :], dacc[:])

        # =========== reduce across cores ===========
        if _sim:
            nc.gpsimd.dma_start(cat_sh[:], cat[:])
        else:
            nc.gpsimd.collective_compute(
                "AllReduce", mybir.AluOpType.add,
                replica_groups=[list(range(N_CORES))],
                ins=[cat[:]], outs=[cat_sh[:]])

        # =========== per-core output projection on its query slice ==========
        tail = pool("tail", 1)
        nred = tail.tile([128, 8, QS], bf16)
        dred = tail.tile([1, HEADS, QS], bf16)
        pid = nc.sync.partition_id()
        qoff = pid * QS
        nc.sync.dma_start(
            nred[:],
            cat_sh[0:DIM, bass.ds(qoff, QS)].rearrange("(h p) q -> p h q", p=128))
        nc.sync.dma_start(dred[:], cat_sh[DIM:DIM + HEADS, bass.ds(qoff, QS)])
        rd = tail.tile([1, HEADS, QS], f32)
        nc.vector.reciprocal(rd[:], dred[:])
        rdb = tail.tile([128, HEADS, QS], f32)
        nc.gpsimd.partition_broadcast(rdb[:], rd[:])
        nsc = tail.tile([128, 8, QS], fp8)
        nc.vector.tensor_mul(nsc[:], nred[:], rdb[:])
        out_sb = tail.tile([128, 8, QS], f32)
        pp_t = pool("pp_t", 2, space="PSUM")
        for e in range(8):
            ps_o = pp_t.tile([128, QS], f32, tag="ppo")
            for pr in range(4):
                nc.tensor.matmul(ps_o[:], wo_s[:, 2 * pr:2 * pr + 2, e, :],
                                 nsc[:, 2 * pr:2 * pr + 2, :],
                                 start=(pr == 0), stop=(pr == 3), perf_mode=DR)
            nc.scalar.activation(out_sb[:, e, :], ps_o[:], AF.Identity,
                                 scale=1.0 / (WS * WS), bias=bo_s[:, e:e + 1])
        nc.sync.dma_start(
            outT.rearrange("(e p) q -> p e q", p=128)[:, :, 0:QS], out_sb[:])

        for cm, _ in reversed(ctx_pools):
            cm.__exit__(None, None, None)

    nc.compile()
    _cache[key] = nc
    _cache["nc"] = nc
    return nc


def _skc_for(nkeep):
    return max(CHT * 128, int(np.ceil(nkeep / (N_CORES * 512))) * 512)


def _prep(x, mem, mask, cos_q, sin_q, cos_k, sin_k,
          Wq, bq, Wk, bk, Wv, bv, Wo, bo, gq, gk):
    import ml_dtypes
    f = np.float32
    bf = ml_dtypes.bfloat16
    f8 = ml_dtypes.float8_e4m3
    x = np.asarray(x, f).reshape(SQ, DIM)
    mem = np.asarray(mem, f).reshape(-1, DIM)
    mask = np.asarray(mask).reshape(-1)
    cos_q = np.asarray(cos_q, f)
    sin_q = np.asarray(sin_q, f)
    cos_k = np.asarray(cos_k, f)
    sin_k = np.asarray(sin_k, f)
    Wq, Wk, Wv, Wo = (np.asarray(w, f) for w in (Wq, Wk, Wv, Wo))
    bq, bk, bv, bo, gq, gk = (np.asarray(v, f) for v in (bq, bk, bv, bo, gq, gk))

    if not np.allclose(gk, 1.0):
        gkp = gk.reshape(-1, 2)
        assert np.allclose(gkp[:, 0], gkp[:, 1]), "unsupported non-pairwise gk"

    idx = np.flatnonzero(mask)
    nkeep = len(idx)
    skc = _skc_for(nkeep)
    base, rem = divmod(nkeep, N_CORES)
    counts = [base + (1 if c < rem else 0) for c in range(N_CORES)]
    offs = np.concatenate([[0], np.cumsum(counts)])

    def tile_w(WT):  # [1024,1024] (in,out of W.T) -> [p, i, o, m], scaled
        return np.ascontiguousarray(
            (WT * WS).reshape(8, 128, 8, 128).transpose(1, 0, 2, 3)).astype(f8)

    ii = np.arange(128)
    jj = ii // 2
    partner = ii ^ 1

    # fold gq (and pairwise gk) into the q rope tables; sin pairs with
    # partner's gq
    gq_t = (gq * gk).reshape(8, 128)
    gq_sin = (gq.reshape(8, 128)[:, partner] * gk.reshape(8, 128))
    cq = cos_q[:, jj].T                # [128, SQ]
    sq = sin_q[:, jj].T
    ctq = np.ascontiguousarray(
        (cq[None, :, :] * gq_t[:, :, None]).transpose(1, 0, 2)).astype(bf)
    stq = np.ascontiguousarray(
        (sq[None, :, :] * gq_sin[:, :, None]).transpose(1, 0, 2)).astype(bf)

    PT = np.zeros((128, 128), f)
    even = ii[ii % 2 == 0]
    PT[even + 1, even] = -1.0
    PT[even, even + 1] = 1.0

    selm = np.zeros((128, 2, 64), f)
    for h in range(8):
        selm[:, :, h * 8 + h] = 1.0

    qones = np.ones((1, 8, SQ), f).astype(f8)

    bo_f = bo + Wo @ bv

    shared = {
        "xt": np.ascontiguousarray(
            x.T.reshape(8, 128, SQ).transpose(1, 0, 2)).astype(f8),
        "wq": tile_w(Wq.T), "wk": tile_w(Wk.T), "wo": tile_w(Wo.T),
        "wv": np.ascontiguousarray(
            (Wv.T * WS).reshape(8, 128, DIM).transpose(1, 0, 2)).astype(f8),
        "ctq": ctq, "stq": stq,
        "bq_t": np.ascontiguousarray(bq.reshape(8, 128).T),
        "bk_t": np.ascontiguousarray(bk.reshape(8, 128).T),
        "bo_t": np.ascontiguousarray(bo_f.reshape(8, 128).T),
        "pmat": PT.astype(bf),
        "ones_c": np.ones((128, 1), bf),
        "sel": selm.astype(f8),
        "qones": qones,
        "eps_c": np.full((1, 1), EPS, f),
        "shf_c": np.full((128, 1), SHIFT, f),
    }

    ckT = cos_k[:, jj].T.astype(f)     # [128, SK]
    skT = sin_k[:, jj].T.astype(f)

    in_maps = []
    for c in range(N_CORES):
        keys = idx[offs[c]:offs[c + 1]]
        n = len(keys)
        m = dict(shared)
        memc = np.zeros((DIM, skc), f8)
        memc[:, :n] = mem[keys].T.astype(f8)
        cstk = np.zeros((HD, 2, skc), bf)
        cstk[:, 0, :n] = ckT[:, keys].astype(bf)
        cstk[:, 1, :n] = skT[:, keys].astype(bf)
        mb = np.full((skc,), MBPAD, f)
        mb[:n] = 0.0
        m["memT"] = memc
        m["cstk"] = cstk
        m["mbr"] = np.ascontiguousarray(
            np.broadcast_to(mb[None, None, :], (1, HEADS, skc))).astype(f8)
        in_maps.append(m)
    return in_maps


def _assemble(outTs):
    parts = [np.asarray(outTs[c])[:, 0:QS].T for c in range(N_CORES)]
    out = np.concatenate(parts, axis=0)
    return out[None].astype(np.float32)


def kernel(**inputs):
    from concourse.bass_utils import run_bass_kernel_spmd
    in_maps = _prep(**inputs)
    skc = in_maps[0]["memT"].shape[1]
    nc = _build(skc)
    res = run_bass_kernel_spmd(nc, in_maps, list(range(N_CORES)))
    return _assemble([res.results[c]["outT"] for c in range(N_CORES)])
